# revision 47
# baseline (speedup 1.0000x reference)
"""AttentionBlock (GroupNorm + 8-head d=64 attention + proj + residual)
for Trainium2, data-parallel over batch across 8 NeuronCores.

v2: ACT-paced pipeline. The softmax exp (Scalar engine, 64 x [128,1024]
per batch) is the hard floor; everything else is packed under it:
 - qkv / PV / proj matmuls in fp8 DoubleRow (half the PE instructions)
 - scores matmuls in f16, consecutive slots alternate PE row-groups
   (auto tile_position) so pairs stream concurrently
 - scores emitted one slot ahead of fillers so exp never waits
 - softmax denominators batched: one [16,512] reciprocal per batch,
   one partition_broadcast, instead of 16 single-lane reciprocals
 - bias/copy work split between GpSimd and DVE
"""
import sys

sys.path.insert(0, "/opt/trn_rl_repo")

import numpy as np
import ml_dtypes

# Problem constants (hardcoded per the task contract).
B, C, HH, WW = 16, 512, 32, 32
N = HH * WW          # 1024 spatial positions
NH, D = 8, 64        # heads, head dim
GROUPS = 8
EPS = 1e-5
NCORES = 8
BPC = B // NCORES    # batches per core
CT = C // 128        # 4 channel tiles
NT = N // 128        # 8 spatial tiles
SCALE2 = float(np.float32(1.0 / np.sqrt(D)) ** 2)  # applied to q AND k
WSCALE = 16.0        # fp8 weights pre-scaled by this; compensated downstream

_RUNNER = None


def _build(n_reps=1, loop_n=None, probe=None):
    import concourse.bacc as bacc
    import concourse.bass as bass
    import concourse.tile as tile
    from concourse import mybir, bass_isa

    f32 = mybir.dt.float32
    f16 = mybir.dt.float16
    f8 = mybir.dt.float8e4
    OP = mybir.AluOpType
    AF = mybir.ActivationFunctionType
    RED = bass_isa.ReduceOp
    DR = mybir.MatmulPerfMode.DoubleRow

    nc = bacc.Bacc("TRN2", target_bir_lowering=False, debug=False,
                   num_devices=NCORES)

    xs = nc.dram_tensor("xs", [BPC, C, N], f16, kind="ExternalInput").ap()
    wqkvT8 = nc.dram_tensor("wqkvT8", [C, 3 * C], f8, kind="ExternalInput").ap()
    wprojT8 = nc.dram_tensor("wprojT8", [C, C], f8, kind="ExternalInput").ap()
    qkvb = nc.dram_tensor("qkvb", [128, 2 * CT], f32, kind="ExternalInput").ap()
    vbias = nc.dram_tensor("vbias", [128, C], f16, kind="ExternalInput").ap()
    projb = nc.dram_tensor("projb", [128, CT], f32, kind="ExternalInput").ap()
    normw = nc.dram_tensor("normw", [128, CT], f32, kind="ExternalInput").ap()
    normb = nc.dram_tensor("normb", [128, CT], f32, kind="ExternalInput").ap()
    ys = nc.dram_tensor("ys", [BPC, C, N], f32, kind="ExternalOutput").ap()

    with tile.TileContext(nc) as tc:
        import contextlib
        ctx = contextlib.ExitStack()
        with ctx:
            consts = ctx.enter_context(tc.tile_pool(name="consts", bufs=1))
            xpool = ctx.enter_context(tc.tile_pool(name="xpool", bufs=2))
            xnpool = ctx.enter_context(tc.tile_pool(name="xnpool", bufs=2))
            qpool = ctx.enter_context(tc.tile_pool(name="qpool", bufs=2))
            kpool = ctx.enter_context(tc.tile_pool(name="kpool", bufs=2))
            vtpool = ctx.enter_context(tc.tile_pool(name="vtpool", bufs=2))
            epool = ctx.enter_context(tc.tile_pool(name="epool", bufs=2))
            pvopool = ctx.enter_context(tc.tile_pool(name="pvopool", bufs=2))
            aopool = ctx.enter_context(tc.tile_pool(name="aopool", bufs=2))
            dpool = ctx.enter_context(tc.tile_pool(name="dpool", bufs=1))
            rbcpool = ctx.enter_context(tc.tile_pool(name="rbcpool", bufs=1))
            outpool = ctx.enter_context(tc.tile_pool(name="outpool", bufs=2))
            tpool = ctx.enter_context(tc.tile_pool(name="tpool", bufs=2))
            stpool = ctx.enter_context(tc.tile_pool(name="stpool", bufs=2))
            sqpool = ctx.enter_context(tc.tile_pool(name="sqpool", bufs=1))
            sppool = ctx.enter_context(
                tc.tile_pool(name="sppool", bufs=2, space="PSUM"))
            pvpool = ctx.enter_context(
                tc.tile_pool(name="pvpool", bufs=2, space="PSUM"))
            fpool = ctx.enter_context(
                tc.tile_pool(name="fpool", bufs=2, space="PSUM"))

            # ---------------- constants / weights ----------------
            wq_sb = consts.tile([128, CT, 3 * C], f8, tag="wq", name="wq")
            wqv = wqkvT8.rearrange("(kt p) o -> p kt o", p=128)
            for kt in range(CT):
                nc.sync.dma_start(out=wq_sb[:, kt, :], in_=wqv[:, kt, :])
            wp_sb = consts.tile([128, CT, C], f8, tag="wp", name="wp")
            wpv = wprojT8.rearrange("(kt p) o -> p kt o", p=128)
            for kt in range(CT):
                nc.sync.dma_start(out=wp_sb[:, kt, :], in_=wpv[:, kt, :])
            qkvb_sb = consts.tile([128, 2 * CT], f32, tag="qkvb", name="qkvb")
            nc.sync.dma_start(out=qkvb_sb, in_=qkvb)
            vbias_sb = consts.tile([128, C], f16, tag="vbias", name="vbias")
            nc.sync.dma_start(out=vbias_sb, in_=vbias)
            projb_sb = consts.tile([128, CT], f32, tag="projb", name="projb")
            nc.sync.dma_start(out=projb_sb, in_=projb)
            nw_sb = consts.tile([128, CT], f32, tag="nw", name="nw")
            nc.sync.dma_start(out=nw_sb, in_=normw)
            nb_sb = consts.tile([128, CT], f32, tag="nb", name="nb")
            nc.sync.dma_start(out=nb_sb, in_=normb)
            eps_sb = consts.tile([128, 1], f32, tag="eps", name="eps")
            nc.vector.memset(eps_sb, EPS)

            # ---------------- per-batch state ----------------
            x_t = [None] * BPC
            xn_t = [None] * BPC
            q_t = [None] * BPC
            k_t = [None] * BPC
            vt_t = [None] * BPC
            e_t = [[None] * 4 for _ in range(BPC)]
            pvo_t = [None] * BPC
            ao_t = [None] * BPC
            dcol_t = [None] * BPC
            dcolr_t = [None] * BPC
            drow_t = [None] * BPC
            rbc_t = [None] * BPC

            dmae = [nc.sync, nc.scalar, nc.gpsimd, nc.sync]

            def load_x(b):
                x_t[b] = xpool.tile([128, CT, N], f16, tag="x", name="x")
                xv = xs[b].rearrange("(ct p) n -> p ct n", p=128)
                for ct_ in range(CT):
                    dmae[ct_].dma_start(out=x_t[b][:, ct_, :],
                                        in_=xv[:, ct_, :])

            def gn(b):
                # per-channel partial sums of x and x^2 over spatial (free)
                sq = stpool.tile([128, 2 * CT], f32, tag="sq", name="sq")
                for ct_ in range(CT):
                    nc.vector.tensor_reduce(
                        out=sq[:, ct_:ct_ + 1], in_=x_t[b][:, ct_, :],
                        axis=mybir.AxisListType.X, op=OP.add)
                    xsq = sqpool.tile([128, N], f16, tag="xsq", name="xsq")
                    nc.vector.scalar_tensor_tensor(
                        out=xsq, in0=x_t[b][:, ct_, :], scalar=1.0,
                        in1=x_t[b][:, ct_, :], op0=OP.mult, op1=OP.mult,
                        accum_out=sq[:, CT + ct_:CT + ct_ + 1])
                # cross-partition sums within each 64-channel group
                sqr = stpool.tile([128, 2 * CT], f32, tag="sqr", name="sqr")
                nc.gpsimd.partition_all_reduce(
                    sqr[0:64, :], sq[0:64, :], 64, RED.add)
                hi = stpool.tile([64, 2 * CT], f32, tag="hi", name="hi")
                nc.sync.dma_start(out=hi, in_=sq[64:128, :])
                nc.gpsimd.partition_all_reduce(hi, hi, 64, RED.add)
                nc.sync.dma_start(out=sqr[64:128, :], in_=hi)
                inv = 1.0 / (64 * N)
                mcol = stpool.tile([128, CT], f32, tag="mcol", name="mcol")
                nc.vector.tensor_scalar_mul(mcol, sqr[:, 0:CT], inv)
                e2col = stpool.tile([128, CT], f32, tag="e2col", name="e2col")
                nc.vector.tensor_scalar_mul(e2col, sqr[:, CT:2 * CT], inv)
                m2col = stpool.tile([128, CT], f32, tag="m2col", name="m2col")
                nc.vector.tensor_mul(m2col, mcol, mcol)
                varcol = stpool.tile([128, CT], f32, tag="varcol", name="varcol")
                nc.vector.tensor_sub(varcol, e2col, m2col)
                lncol = stpool.tile([128, CT], f32, tag="lncol", name="lncol")
                nc.scalar.activation(lncol, varcol, AF.Ln, bias=eps_sb)
                rscol = stpool.tile([128, CT], f32, tag="rscol", name="rscol")
                nc.scalar.activation(rscol, lncol, AF.Exp, scale=-0.5)
                acol = stpool.tile([128, CT], f32, tag="acol", name="acol")
                nc.vector.tensor_mul(acol, rscol, nw_sb)
                macol = stpool.tile([128, CT], f32, tag="macol", name="macol")
                nc.vector.tensor_mul(macol, mcol, acol)
                bcol = stpool.tile([128, CT], f32, tag="bcol", name="bcol")
                nc.vector.tensor_sub(bcol, nb_sb, macol)
                xn_t[b] = xnpool.tile([128, CT, N], f8, tag="xn", name="xn")
                for ct_ in range(CT):
                    nc.vector.tensor_scalar(
                        out=xn_t[b][:, ct_, :], in0=x_t[b][:, ct_, :],
                        scalar1=acol[:, ct_:ct_ + 1],
                        scalar2=bcol[:, ct_:ct_ + 1],
                        op0=OP.mult, op1=OP.add)

            def qkv_alloc(b):
                q_t[b] = qpool.tile([128, CT, N], f16, tag="q", name="q")
                k_t[b] = kpool.tile([128, CT, N], f16, tag="k", name="k")
                # vt: [p, pair-elem i, pair u, head, D + denom-ones + pad]
                vt_t[b] = vtpool.tile([128, 2, 4, NH, D + 2], f8, tag="vt",
                                      name="vt")
                # the "ones" column carries WSCALE so the denominator comes
                # out pre-multiplied (compensates fp8 weight scaling in proj)
                nc.gpsimd.memset(vt_t[b][:, :, :, :, D:D + 1], WSCALE)
                nc.gpsimd.memset(vt_t[b][:, :, :, :, D + 1:D + 2], 0.0)

            def qkv_qk(b, mt, nch):
                """mt 0..7 (0-3 -> q pair-blocks, 4-7 -> k), nch 0..1."""
                ps = fpool.tile([128, 512], f32, tag="fp", name="fp")
                for kp in range(2):
                    nc.tensor.matmul(
                        ps,
                        lhsT=wq_sb[:, 2 * kp:2 * kp + 2,
                                   mt * 128:(mt + 1) * 128],
                        rhs=xn_t[b][:, 2 * kp:2 * kp + 2,
                                    nch * 512:(nch + 1) * 512],
                        start=(kp == 0), stop=(kp == 1), perf_mode=DR)
                dst = q_t[b] if mt < CT else k_t[b]
                nc.vector.tensor_scalar(
                    out=dst[:, mt % CT, nch * 512:(nch + 1) * 512], in0=ps,
                    scalar1=1.0 / WSCALE, scalar2=qkvb_sb[:, mt:mt + 1],
                    op0=OP.mult, op1=OP.add)

            def qkv_v(b, nt):
                ps = fpool.tile([128, 512], f32, tag="fp", name="fp")
                for kp in range(2):
                    nc.tensor.matmul(
                        ps,
                        lhsT=xn_t[b][:, 2 * kp:2 * kp + 2,
                                     nt * 128:(nt + 1) * 128],
                        rhs=wq_sb[:, 2 * kp:2 * kp + 2, 2 * C:3 * C],
                        start=(kp == 0), stop=(kp == 1), perf_mode=DR)
                psv = ps.rearrange("p (h d) -> p h d", h=NH)
                vbv = vbias_sb.rearrange("p (h d) -> p h d", h=NH)
                nc.vector.scalar_tensor_tensor(
                    out=vt_t[b][:, nt % 2, nt // 2, :, 0:D], in0=psv,
                    scalar=1.0 / WSCALE, in1=vbv, op0=OP.mult, op1=OP.add)

            def warm_mm(ps, free=512):
                """Discardable MM to keep the PE HAM clock-gate warm: the
                next real MM into ps has start=True, which clears the
                has_written bits and overwrites this result entirely."""
                p = ps.shape[0]
                nc.tensor.matmul(ps[:, 0:free], lhsT=wq_sb[:, 0, 0:p],
                                 rhs=wq_sb[:, 0, 0:free],
                                 start=True, stop=True)

            def scores_mms(b, j, mt):
                """Both heads of (j, mt): 4 MMs issued back-to-back,
                h0 rows 0-63 first so the next superslot's first exp is
                never blocked behind the h1 psum WAR."""
                sp = [sppool.tile([128, N], f32, tag="sp", name="sp")
                      for _ in range(2)]
                for hp in range(2):
                    par = slice(hp * 64, hp * 64 + 64)
                    for nch in range(2):
                        nc.tensor.matmul(
                            sp[hp][:, nch * 512:(nch + 1) * 512],
                            lhsT=k_t[b][par, j, mt * 128:(mt + 1) * 128],
                            rhs=q_t[b][par, j, nch * 512:(nch + 1) * 512],
                            start=True, stop=True)
                return sp

            def exp_slot(b, j, mt, h, sp):
                nc.scalar.activation(
                    e_t[b][j][:, mt % 2, mt // 2, h % 2, :], sp, AF.Exp,
                    scale=SCALE2)

            def pv_chunk(b, j, h, nch):
                pvp = pvpool.tile([66, 512], f32, tag="pv", name="pv")
                ej = e_t[b][j]
                for u in range(4):
                    nc.tensor.matmul(
                        pvp,
                        lhsT=vt_t[b][:, :, u, h, :],
                        rhs=ej[:, :, u, h % 2, nch * 512:(nch + 1) * 512],
                        start=(u == 0), stop=(u == 3), perf_mode=DR)
                idx = h * 2 + nch
                nc.vector.tensor_copy(out=pvo_t[b][0:65, idx, :],
                                      in_=pvp[0:65, :])
                nc.scalar.dma_start(
                    out=dcol_t[b][idx // 8][idx % 8:idx % 8 + 1, :],
                    in_=pvo_t[b][64:65, idx, :])

            def norm_half(b, half):
                """Reciprocal + broadcast for denominator chunks
                idx half*8 .. half*8+7 (available a pair earlier than the
                full set, so half 0 runs inside the same attention)."""
                with nc.allow_low_precision(reason="softmax recip in f16"):
                    nc.vector.reciprocal(dcolr_t[b][half],
                                         dcol_t[b][half])
                for i in range(8):
                    idx = half * 8 + i
                    dmae[i % 4].dma_start(
                        out=drow_t[b][0:1, idx * 512:(idx + 1) * 512],
                        in_=dcolr_t[b][half][i:i + 1, :])
                if half == 0:
                    rbc_t[b] = rbcpool.tile([64, 16 * 512], f16, tag="rbc",
                                            name="rbc")
                nc.gpsimd.partition_broadcast(
                    rbc_t[b][:, half * 4096:(half + 1) * 4096],
                    drow_t[b][0:1, half * 4096:(half + 1) * 4096],
                    channels=64)

            def norm_mul(b, idx):
                h, nch = idx // 2, idx % 2
                ct_ = h // 2
                dsl = slice(nch * 512, (nch + 1) * 512)
                rsl = rbc_t[b][0:64, idx * 512:(idx + 1) * 512]
                if h % 2 == 0:
                    nc.vector.tensor_mul(ao_t[b][0:64, ct_, dsl],
                                         pvo_t[b][0:64, idx, :], rsl)
                else:
                    tmp = tpool.tile([64, 512], f8, tag="tmp", name="tmp")
                    nc.vector.tensor_mul(tmp, pvo_t[b][0:64, idx, :], rsl)
                    nc.sync.dma_start(out=ao_t[b][64:128, ct_, dsl], in_=tmp)

            def proj_unit(b, u):
                mt, nch = u // 2, u % 2
                ps = fpool.tile([128, 512], f32, tag="fp", name="fp")
                for kp in range(2):
                    nc.tensor.matmul(
                        ps,
                        lhsT=wp_sb[:, 2 * kp:2 * kp + 2,
                                   mt * 128:(mt + 1) * 128],
                        rhs=ao_t[b][:, 2 * kp:2 * kp + 2,
                                    nch * 512:(nch + 1) * 512],
                        start=(kp == 0), stop=(kp == 1), perf_mode=DR)
                ot = outpool.tile([128, 512], f32, tag="out", name="out")
                nc.vector.scalar_tensor_tensor(
                    out=ot, in0=ps, scalar=projb_sb[:, mt:mt + 1],
                    in1=x_t[b][:, mt, nch * 512:(nch + 1) * 512],
                    op0=OP.add, op1=OP.add)
                nc.sync.dma_start(
                    out=ys[b, mt * 128:(mt + 1) * 128,
                           nch * 512:(nch + 1) * 512],
                    in_=ot)

            def attention(b, buckets):
                """64 slots of (pair j, mt, head). Scores are emitted one
                slot ahead of exp so ACT never waits on the PE queue.
                buckets: 4 lists of filler callables; bucket j is fully
                drained within pair j's 16 slots (deadline scheduling —
                emission order must respect producer-before-consumer).
                Returns the tail carry (pair-3 pv + normalize + proj)."""
                ao_t[b] = aopool.tile([128, CT, N], f8, tag="ao", name="ao")
                pvo_t[b] = pvopool.tile([66, 16, 512], f16, tag="pvo",
                                        name="pvo")
                dcol_t[b] = [dpool.tile([8, 512], f16, tag=f"dcol{i}",
                                        name="dcol") for i in range(2)]
                dcolr_t[b] = [dpool.tile([8, 512], f16, tag=f"dcolr{i}",
                                         name="dcolr") for i in range(2)]
                drow_t[b] = dpool.tile([1, 16 * 512], f16, tag="drow",
                                       name="drow")
                slots = [(j, mt) for j in range(4) for mt in range(NT)]
                nslots = len(slots)
                sps = {}
                pvq = []
                drained = [0, 0, 0, 0]

                def emit_scores(si):
                    j, mt = slots[si]
                    if mt == 0:
                        e_t[b][j] = epool.tile([128, 2, 4, 2, N], f8,
                                               tag="e", name="e")
                    sps[si] = scores_mms(b, j, mt)

                # normalize half 0 (heads 0-3, available after pair 2)
                # runs as pair-3 fillers of this same attention window
                buckets = list(buckets)
                buckets[3] = buckets[3] + [lambda: norm_half(b, 0)] + \
                    [lambda i=idx: norm_mul(b, i) for idx in range(8)]

                emit_scores(0)
                for si, (j, mt) in enumerate(slots):
                    sp = sps.pop(si)
                    for hp in range(2):
                        exp_slot(b, j, mt, 2 * j + hp, sp[hp])
                    # pv chunks of the previous pair: 1 per 2 superslots
                    # (popped before the lookahead emission so epool bufs=2
                    # reuse keeps producer-before-consumer emission order;
                    # on the strict-FIFO PE the likely-blocked scores MMs
                    # come last so pv/filler work absorbs the exp latency)
                    if pvq and si % 2 == 0:
                        pvq.pop(0)()
                    # bucket fillers, paced evenly; fully drained by
                    # superslot 6 so the pair-(j+1) lookahead (emitted at
                    # superslot 7) sees its k/q producers emitted
                    bq = buckets[j]
                    sl = si % 8 + 1
                    want = len(bq) if sl >= 7 else min(
                        len(bq) * sl // 6, len(bq))
                    while drained[j] < want:
                        bq[drained[j]]()
                        drained[j] += 1
                    # lookahead 1: the 4 MMs WAR-wait this superslot's exps,
                    # which retire while the fillers above run
                    if si + 1 < nslots:
                        emit_scores(si + 1)
                    if mt == NT - 1:
                        for hp in range(2):
                            for nch in range(2):
                                pvq.append(
                                    lambda jj=j, hh=2 * j + hp, nn=nch:
                                    pv_chunk(b, jj, hh, nn))
                carry = list(pvq)
                carry.append(lambda: norm_half(b, 1))
                for idx in range(8, 16):
                    carry.append(lambda i=idx: norm_mul(b, i))
                return carry

            # ---------------- emission ----------------
            def qk_units(b, pairs):
                return [lambda a=(mt_, nch_): qkv_qk(b, *a)
                        for mt_ in pairs for nch_ in range(2)]

            def v_units(b, nts):
                return [lambda nt=nt_: qkv_v(b, nt) for nt_ in nts]

            def emit_rep(carry_in):
                """One rep (2 batches). carry_in: the previous rep's
                batch-1 tail (pv pair3 + normalize half + muls + proj),
                drained inside attention(0)'s first pair so it overlaps
                this rep's loads/gn instead of being exposed serially at
                the rep boundary. Returns this rep's batch-1 tail.
                gn(0) is emitted before the carry so its DVE work runs
                ahead of the carry's muls in the DVE queue."""
                load_x(0)
                gn(0)
                qkv_alloc(0)
                for mt, nch in ((0, 0), (0, 1), (4, 0), (4, 1)):
                    qkv_qk(0, mt, nch)

                def gn1():
                    gn(1)
                    qkv_alloc(1)

                # deadline buckets: pair-j scores (lookahead slot 16j-1)
                # need k/q[j]; pair-j pv (during pair j+1) needs all of vt.
                # load_x(1) sits after the carry (its proj reads old x(1)).
                b0_0 = list(carry_in) + [lambda: load_x(1)] + \
                    v_units(0, range(8)) + qk_units(0, (1, 5))
                b0_1 = [gn1] + qk_units(0, (2, 6))
                b0_2 = qk_units(0, (3, 7)) + qk_units(1, (0, 4))
                b0_3 = v_units(1, range(8)) + qk_units(1, (1, 5))
                c0 = attention(0, [b0_0, b0_1, b0_2, b0_3])
                # batch-0 tail (pv pair3 + normalize half 1) lands at the
                # head of attention(1); proj(0) follows in pairs 1-2 once
                # all of ao(0) is normalized
                b1_0 = c0
                b1_1 = qk_units(1, (2, 6)) + \
                    [lambda u=u_: proj_unit(0, u) for u_ in range(4)]
                b1_2 = qk_units(1, (3, 7)) + \
                    [lambda u=u_: proj_unit(0, u) for u_ in range(4, 8)]
                c1 = attention(1, [b1_0, b1_1, b1_2, []])
                return c1 + [lambda u=u_: proj_unit(1, u) for u_ in range(8)]

            # Software-pipelined across reps. The Tile scheduler cannot
            # wire read-before-write loop-carried deps inside a For_i
            # body, so the loop is unrolled 4 reps per body: the three
            # internal rep boundaries pipeline as ordinary forward deps
            # (carry drained inside the next rep's attention window);
            # only the body-to-body boundary drains serially.
            if loop_n is not None:
                assert loop_n % 8 == 0
                with tc.For_i(0, loop_n // 8, 1):
                    cy = []
                    for _u in range(8):
                        cy = emit_rep(cy)
                    for f in cy:
                        f()
            else:
                cy = []
                for _rep in range(n_reps):
                    cy = emit_rep(cy)
                for f in cy:
                    f()

    nc.compile()
    return nc


def _make_runner():
    """Build the bass program once and return a cached callable
    (list of per-core input dicts) -> list of per-core output dicts."""
    import jax
    from jax.experimental.shard_map import shard_map
    from jax.sharding import Mesh, PartitionSpec
    from concourse import mybir
    from concourse import bass2jax

    nc = _build()
    bass2jax.install_neuronx_cc_hook()

    partition_name = (nc.partition_id_tensor.name
                      if nc.partition_id_tensor else None)
    in_names, out_names, out_avals, zero_outs = [], [], [], []
    for alloc in nc.m.functions[0].allocations:
        if not isinstance(alloc, mybir.MemoryLocationSet):
            continue
        name = alloc.memorylocations[0].name
        if alloc.kind == "ExternalInput":
            if name != partition_name:
                in_names.append(name)
        elif alloc.kind == "ExternalOutput":
            shape = tuple(alloc.tensor_shape)
            dtype = mybir.dt.np(alloc.dtype)
            out_names.append(name)
            out_avals.append(jax.core.ShapedArray(shape, dtype))
            zero_outs.append(np.zeros(shape, dtype))
    n_params = len(in_names)
    n_outs = len(out_avals)
    all_names = in_names + out_names
    if partition_name is not None:
        all_names = all_names + [partition_name]
    donate = tuple(range(n_params, n_params + n_outs))

    def _body(*args):
        operands = list(args)
        if partition_name is not None:
            operands.append(bass2jax.partition_id_tensor())
        outs = bass2jax._bass_exec_p.bind(
            *operands,
            out_avals=tuple(out_avals),
            in_names=tuple(all_names),
            out_names=tuple(out_names),
            lowering_input_output_aliases=(),
            sim_require_finite=True,
            sim_require_nnan=True,
            nc=nc,
        )
        return tuple(outs)

    devices = jax.devices()[:NCORES]
    mesh = Mesh(np.asarray(devices), ("core",))
    in_specs = (PartitionSpec("core"),) * (n_params + n_outs)
    out_specs = (PartitionSpec("core"),) * n_outs
    sharded = jax.jit(
        shard_map(_body, mesh=mesh, in_specs=in_specs, out_specs=out_specs,
                  check_rep=False),
        donate_argnums=donate, keep_unused=True)

    def run(in_maps):
        concat_in = [
            np.concatenate([np.asarray(in_maps[c][nm]) for c in range(NCORES)],
                           axis=0)
            for nm in in_names
        ]
        concat_zeros = [
            np.zeros((NCORES * z.shape[0], *z.shape[1:]), z.dtype)
            for z in zero_outs
        ]
        out_arrs = sharded(*concat_in, *concat_zeros)
        return [
            {nm: np.asarray(out_arrs[i]).reshape(NCORES, *out_avals[i].shape)[c]
             for i, nm in enumerate(out_names)}
            for c in range(NCORES)
        ]

    return run


def _prep_consts(norm_w, norm_b, qkv_w, qkv_b, proj_w, proj_b):
    f8 = ml_dtypes.float8_e4m3
    wqkvT8 = np.ascontiguousarray(qkv_w.T * WSCALE).astype(f8)
    wprojT8 = np.ascontiguousarray(proj_w.T * WSCALE).astype(f8)
    qkvb = np.ascontiguousarray(
        qkv_b[:2 * C].reshape(2 * CT, 128).T).astype(np.float32)
    vbias = np.ascontiguousarray(
        np.broadcast_to(qkv_b[2 * C:3 * C], (128, C))).astype(np.float16)
    projb = np.ascontiguousarray(
        proj_b.reshape(CT, 128).T).astype(np.float32)
    normw = np.ascontiguousarray(
        norm_w.reshape(CT, 128).T).astype(np.float32)
    normb = np.ascontiguousarray(
        norm_b.reshape(CT, 128).T).astype(np.float32)
    return dict(wqkvT8=wqkvT8, wprojT8=wprojT8, qkvb=qkvb, vbias=vbias,
                projb=projb, normw=normw, normb=normb)


def kernel(x, norm_w, norm_b, qkv_w, qkv_b, proj_w, proj_b, num_heads):
    global _RUNNER
    assert num_heads == NH
    x = np.asarray(x, dtype=np.float32)
    consts = _prep_consts(np.asarray(norm_w), np.asarray(norm_b),
                          np.asarray(qkv_w), np.asarray(qkv_b),
                          np.asarray(proj_w), np.asarray(proj_b))
    xsr = x.reshape(B, C, N).astype(np.float16)
    in_maps = [
        {"xs": np.ascontiguousarray(xsr[c * BPC:(c + 1) * BPC]), **consts}
        for c in range(NCORES)
    ]
    if _RUNNER is None:
        _RUNNER = _make_runner()
    results = _RUNNER(in_maps)
    out = np.concatenate([results[c]["ys"] for c in range(NCORES)], axis=0)
    return out.reshape(B, C, HH, WW).astype(np.float32)
